# revision 15
# baseline (speedup 1.0000x reference)
"""Bouc-Wen net kernel for Trainium2, 8-core data parallel.

Math (reference):
    dx     = x - x_pre
    alpha  = (a1*e^{2dx} + a2) / (e^{2dx} + 1)  ==  a2 + (a1-a2)*sigmoid(2dx)
    m      = |dx|
    a      = 1 / (1 + v*m)
    c      = (A*dx + delta*m) * a
    h_t    = a_t * h_{t-1} + c_t        (scan over time, per batch row)
    out    = alpha*x + h + b
    returns (out, weights, h_final)

With the actual seeded weights v < 0, so 1+v*m crosses zero and the
recurrence diverges: ~98% of the reference output saturates to +-inf in
fp32.  The kernel mirrors the reference's fp32 rounding sequence so the
inf pattern and signs match exactly; `a` uses the ~51-ULP fast reciprocal
(validated offline: 0 inf-pattern mismatches, ~4e-3 scale-relative finite
error vs the jax reference).

Sharding: batch rows split evenly across 8 cores; the recurrence runs along
the time (free) axis with `tensor_tensor_scan`; the 6 scalar weights are
baked into the program as immediates.

Engine balance per [128 x F] tile (in-place aliasing noted; measured on
HW: full kernel ~176 us/core vs ~170 us pure-DMA floor for the 48 MiB of
traffic -- memory-bound):
    GPSIMD: d = x - x_pre                 (chain head; GPSIMD ~1.45 ns/elem)
    ACT:    sg = sigmoid(2d); m = |d|; q = v*m+1 (on m);
            wv = k*sg+a2 (on sg); carry = h[:,F-1] copy
    DVE:    a = recip_fast(q); c = (A*d + delta*|d|)*a (on d);
            h = scan(a, c); u = wv*x (on wv); o = (h+b)+u (on h)
"""

import sys

if "/opt/trn_rl_repo" not in sys.path:
    sys.path.insert(0, "/opt/trn_rl_repo")

import numpy as np

B, T = 4096, 8192
N_CORES = 8
R = B // N_CORES  # rows per core
P = 128           # SBUF partitions
F = 2048          # time-tile (free dim) size

_CACHE = {}
_BW_C_OP = None


def _get_bw_c_op():
    """Register (once) a custom DVE op computing
    (Src0*C0 + max(Src0,-Src0)*C1) * Src1 with per-stage fp32 rounding --
    exactly the reference's (A*dx + delta*|dx|) * a rounding sequence."""
    global _BW_C_OP
    if _BW_C_OP is not None:
        return _BW_C_OP
    from concourse import dve_ops
    from concourse.dve_spec import Spec, Src0, Src1, C0, C1, lower, maxx
    from concourse.dve_spec import _has_src1 as has_src1
    from concourse.dve_uop import DveOpSpec
    from concourse.dve_ops import DveOp, _CUSTOM_DVE_ROW_BASE

    name = "BW_C_ANT"
    for o in dve_ops.OPS:
        if o.name == name:
            _BW_C_OP = o
            return o

    body = (Src0 * C0 + maxx(Src0, -Src0) * C1) * Src1
    spec = Spec(
        body=body,
        reference=lambda in0, in1, s0, s1, imm2: (
            (in0 * s0 + np.maximum(in0, -in0) * s1) * in1
        ).astype(np.float32),
    )
    opcode = _CUSTOM_DVE_ROW_BASE + len(dve_ops.OPS)
    shas = {}
    for ver in ("v3", "v4"):
        s = DveOpSpec(name=name, opcode=opcode, uops=lower(spec, ver=ver),
                      rd1_en=has_src1(spec))
        shas[ver] = s.sha(ver)
    op = DveOp(name, spec, False, shas)
    dve_ops.OPS.append(op)
    dve_ops._SUB_OPCODE_FOR_NAME[name] = opcode
    dve_ops.CUSTOM_DVE_SPECS[name] = spec
    _BW_C_OP = op
    return op


def _build_program(w, R=R, T=T, F=F, n_cores=N_CORES, rep=1,
                   u_engine="dve", d_engine="gpsimd", io_bufs=3, wk_bufs=2,
                   hp_bufs=2):
    import concourse.bacc as bacc
    import concourse.mybir as mybir
    from concourse import tile

    G = R // P
    J = T // F
    alpha1, alpha2, A, v, delta, b = (float(np.float32(w[i])) for i in range(6))
    k = float(np.float32(np.float32(alpha1) - np.float32(alpha2)))

    f32 = mybir.dt.float32
    Op = mybir.AluOpType
    Act = mybir.ActivationFunctionType
    bw_c = _get_bw_c_op()

    nc = bacc.Bacc("TRN2", target_bir_lowering=False, debug=False,
                   num_devices=n_cores)
    x_d = nc.dram_tensor("x", [R, T], f32, kind="ExternalInput")
    xp_d = nc.dram_tensor("x_pre", [R, T], f32, kind="ExternalInput")
    h0_d = nc.dram_tensor("h_init", [R, 1], f32, kind="ExternalInput")
    out_d = nc.dram_tensor("out", [R, T], f32, kind="ExternalOutput")
    hl_d = nc.dram_tensor("h_last", [R, 1], f32, kind="ExternalOutput")

    with tile.TileContext(nc) as tc:
        with tc.tile_pool(name="io", bufs=io_bufs) as io, \
             tc.tile_pool(name="wk", bufs=wk_bufs) as wk, \
             tc.tile_pool(name="hp", bufs=hp_bufs) as hp, \
             tc.tile_pool(name="carry", bufs=2) as cp:
            for r in range(rep):
                carry = []
                for g in range(G):
                    t0 = cp.tile([P, 1], f32, tag=f"carry_{g}")
                    nc.sync.dma_start(t0[:], h0_d[g * P:(g + 1) * P, :])
                    carry.append(t0)

                for j in range(J):
                    for g in range(G):
                        rs = slice(g * P, (g + 1) * P)
                        cs = slice(j * F, (j + 1) * F)

                        x_t = io.tile([P, F], f32, tag="x")
                        xp_t = io.tile([P, F], f32, tag="xp")
                        nc.sync.dma_start(x_t[:], x_d[rs, cs])
                        nc.sync.dma_start(xp_t[:], xp_d[rs, cs])

                        # d = x - x_pre
                        d_t = wk.tile([P, F], f32, tag="d")
                        if d_engine == "gpsimd":
                            nc.gpsimd.tensor_tensor(d_t[:], x_t[:], xp_t[:],
                                                    Op.subtract)
                        else:
                            nc.vector.tensor_tensor(d_t[:], x_t[:], xp_t[:],
                                                    Op.subtract)

                        # sg = sigmoid(2d)                (ACT)
                        sg_t = wk.tile([P, F], f32, tag="sg")
                        nc.scalar.activation(sg_t[:], d_t[:], Act.Sigmoid,
                                             scale=2.0)

                        # m = |d|                         (ACT)
                        m_t = wk.tile([P, F], f32, tag="m")
                        nc.scalar.activation(m_t[:], d_t[:], Act.Abs)

                        # q = v*m + 1   (in-place on m)   (ACT)
                        nc.scalar.activation(m_t[:], m_t[:], Act.Copy,
                                             bias=1.0, scale=v)

                        # a = 1/q                         (DVE, ~51 ULP)
                        a_t = wk.tile([P, F], f32, tag="a")
                        nc.vector.reciprocal_approx_fast(out=a_t[:],
                                                         in_=m_t[:])

                        # c = (A*d + delta*|d|)*a  (in-place on d) (DVE)
                        nc.vector._custom_dve(bw_c, out=d_t[:], in0=d_t[:],
                                              in1=a_t[:], s0=A, s1=delta)

                        # h = scan(a, c)                  (DVE)
                        h_t = hp.tile([P, F], f32, tag=f"h_{g}")
                        nc.vector.tensor_tensor_scan(h_t[:], a_t[:], d_t[:],
                                                     carry[g][:, 0:1],
                                                     Op.mult, Op.add)
                        nxt = cp.tile([P, 1], f32, tag=f"carry_{g}")
                        nc.scalar.activation(nxt[:], h_t[:, F - 1:F],
                                             Act.Copy)
                        carry[g] = nxt

                        # wv = k*sg + alpha2  (in-place on sg) (ACT)
                        nc.scalar.activation(sg_t[:], sg_t[:], Act.Copy,
                                             bias=alpha2, scale=k)

                        # u = wv * x    (in-place on wv)
                        if u_engine == "gpsimd" or (
                                u_engine == "alt" and (j + g) % 2 == 0):
                            nc.gpsimd.tensor_tensor(sg_t[:], sg_t[:],
                                                    x_t[:], Op.mult)
                        else:
                            nc.vector.tensor_tensor(sg_t[:], sg_t[:],
                                                    x_t[:], Op.mult)

                        # o = (h + b) + u  (in-place on h) (DVE)
                        nc.vector.scalar_tensor_tensor(h_t[:], h_t[:], b,
                                                       sg_t[:], Op.add,
                                                       Op.add)

                        nc.sync.dma_start(out_d[rs, cs], h_t[:])
                        if j == J - 1:
                            nc.sync.dma_start(hl_d[rs, :], nxt[:])

    nc.compile()
    return nc


def _get_program(w):
    key = np.asarray(w, dtype=np.float32).tobytes()
    if key not in _CACHE:
        _CACHE[key] = _build_program(w)
    return _CACHE[key]


def kernel(x, x_pre, h_init, weights):
    from concourse.bass_utils import run_bass_kernel_spmd

    x = np.ascontiguousarray(np.asarray(x, dtype=np.float32).reshape(B, T))
    x_pre = np.ascontiguousarray(
        np.asarray(x_pre, dtype=np.float32).reshape(B, T))
    h_init = np.ascontiguousarray(
        np.asarray(h_init, dtype=np.float32).reshape(B, 1))
    weights = np.asarray(weights, dtype=np.float32).reshape(6)

    nc = _get_program(weights)

    in_maps = []
    for i in range(N_CORES):
        rs = slice(i * R, (i + 1) * R)
        in_maps.append({
            "x": x[rs],
            "x_pre": x_pre[rs],
            "h_init": h_init[rs],
        })

    # Occasional transient NRT_EXEC_UNIT_UNRECOVERABLE crashes have been
    # observed on this fabric; retry a couple of times before giving up.
    import time as _time
    last_err = None
    res = None
    for _attempt in range(3):
        try:
            res = run_bass_kernel_spmd(nc, in_maps, list(range(N_CORES)))
            break
        except Exception as e:  # noqa: BLE001
            last_err = e
            _time.sleep(10)
    if res is None:
        raise last_err

    out = np.empty((B, T), dtype=np.float32)
    h_last = np.empty((B, 1), dtype=np.float32)
    for i in range(N_CORES):
        rs = slice(i * R, (i + 1) * R)
        out[rs] = res.results[i]["out"]
        h_last[rs] = res.results[i]["h_last"]

    return (out.reshape(B, T, 1), weights,
            h_last.reshape(B, 1, 1).astype(np.float32))
